# revision 35
# baseline (speedup 1.0000x reference)
"""Trainium2 Bass kernel for CausalDownsamplingLRU.

Algorithm (per core = one batch element; 8 cores, data-parallel over batch):
  With lam = r*e^{i theta} (per state n), h_t = lam*h_{t-1} + Bu_t, and only
  y[:, -DS:] needed:

  1. Input GEMMs (fp16, PE): Bu^T[n,t] = (gamma*B)^T.T @ x^T  (re & im planes)
  2. FIRST half: only h_{1023} (the carry into the output window) is needed,
     truncated to the last WCON=512 steps (error <= 0.99^512 ~ 0.6%):
        w[i,n]  = sum_s x_s[i] V[s,n],  V = lam^{511-s}     (PE, flipped GEMM)
        a[n]    = sum_i (gamma*B)[n,i] (.) w[i,n]           (PE diag-block MMs
                  into PSUM [n_p, n_f]; diagonal extracted by a masked
                  STT-reduce against an identity tile)
  3. SECOND half: phase twist e_j = e^{-i j theta} (.) Bu_{1024+j} decouples
     the complex recurrence into two REAL per-partition scans
        s_j = r*s_{j-1} + e_j   (tensor_tensor_scan, fp32 state),
     with initial s_{-1} = e^{i theta} * h_1023.
  4. Untwist h = e^{+i j theta} (.) s, then output GEMMs:
        y^T = C_re^T.T @ h_re + (-C_im^T).T @ h_im + D^T.T @ x^T

All elementwise work stays on the DVE (GPSIMD compute shares the DVE SBUF
port: measured ~4x mutual slowdown, so it only issues DMAs here).  Weights
are shipped nb-blocked (btrP) so each state-block's pipeline starts as soon
as its own 0.25MB lands; the 2MB trig tables get a dedicated DMA path since
they pace the twists.
"""
import numpy as np

import concourse.bass as bass
import concourse.bacc as bacc
import concourse.mybir as mybir
from concourse.tile import TileContext
from concourse.bass_utils import run_bass_kernel_spmd

BATCH, T, IN, OUT, N = 8, 2048, 512, 512, 512
DS = 1024
P = 128
NB = N // P    # 4 state blocks
IBN = IN // P  # 4 input blocks
OBN = OUT // P # 4 output blocks
HF = 1024      # window length (= DS)
HH = 512       # half length (PSUM bank limit for f32 out)
WCON = 512     # carry W-GEMM contraction (last WCON steps of first half)
WB = WCON // P

f32 = mybir.dt.float32
f16 = mybir.dt.float16
AOP = mybir.AluOpType

_CACHE = {}


def _build_nc():
    if "nc" in _CACHE:
        return _CACHE["nc"]
    nc = bacc.Bacc()
    xT = nc.dram_tensor("xT", [IN, HF], f16, kind="ExternalInput")    # 2nd half, [i, t]
    xw = nc.dram_tensor("xw", [WCON, IN], f16, kind="ExternalInput")  # x[512:1024], [s, i]
    # nb-blocked weights: [p, nb*IN] with cols (nb, ib, n-within-block)
    btrP = nc.dram_tensor("btrP", [P, NB * IN], f16, kind="ExternalInput")
    btiP = nc.dram_tensor("btiP", [P, NB * IN], f16, kind="ExternalInput")
    vre = nc.dram_tensor("vre", [WCON, N], f16, kind="ExternalInput")
    vim = nc.dram_tensor("vim", [WCON, N], f16, kind="ExternalInput")
    cosj = nc.dram_tensor("cosj", [N, HF], f16, kind="ExternalInput")
    sinj = nc.dram_tensor("sinj", [N, HF], f16, kind="ExternalInput")
    eye = nc.dram_tensor("eye", [P, P], f16, kind="ExternalInput")
    rb = nc.dram_tensor("rb", [N, 1], f32, kind="ExternalInput")
    # rot columns: 0=cos(theta), 1=-sin(theta), 2=sin(theta)
    rot = nc.dram_tensor("rot", [N, 3], f32, kind="ExternalInput")
    ctr = nc.dram_tensor("ctr", [N, OUT], f16, kind="ExternalInput")
    ctin = nc.dram_tensor("ctin", [N, OUT], f16, kind="ExternalInput")
    dtw = nc.dram_tensor("dtw", [IN, OUT], f16, kind="ExternalInput")
    yT = nc.dram_tensor("yT", [OUT, DS], f16, kind="ExternalOutput")

    with TileContext(nc) as tc:
        with (
            tc.tile_pool(name="const", bufs=1) as cp,
            tc.tile_pool(name="work", bufs=1) as wkp,
            tc.tile_pool(name="ps", bufs=3, space="PSUM") as bp,
            tc.tile_pool(name="yps", bufs=5, space="PSUM") as yp,
        ):
            def row_tiles(dram, eng, pfx, cols=None):
                rows = dram.shape[0]
                cols = cols if cols is not None else dram.shape[1]
                out = []
                for i in range(rows // P):
                    t = cp.tile([P, cols], dram.dtype, tag=f"{pfx}{i}", name=f"{pfx}{i}")
                    eng.dma_start(t[:], dram[i * P:(i + 1) * P, 0:cols])
                    out.append(t)
                return out

            # --- DMA map.  Each issue-queue delivers ~90GB/s SERIALLY; the
            # gpsimd SWDGE descriptor writes contend with the DVE SBUF port
            # (+20% DVE time measured), so ALL gpsimd loads are consolidated
            # and issued up-front to finish before the vector phase; scalar
            # (the evac engine) only carries pre-first-evac loads; sync
            # carries the trig tables (they pace the twists) + late weights.
            def load_consolidated(dram, eng, tagp, cols=None, col0=0):
                """One rearranged DMA for a [R*P, C] dram table -> row tiles."""
                rows = dram.shape[0]
                nb_ = rows // P
                cols = cols if cols is not None else dram.shape[1]
                big = cp.tile([P, nb_ * cols], dram.dtype, tag=tagp, name=tagp)
                eng.dma_start(
                    big[:].rearrange("p (b c) -> p b c", b=nb_),
                    dram[:, col0:col0 + cols].rearrange("(b p) c -> p b c", p=P))
                return [big[:, i * cols:(i + 1) * cols] for i in range(nb_)]

            # sync gets x-h0 FIRST (it gates bu0 -> the first twist); its trig
            # follows.  gpsimd: x h1 tail + weights + w-gemm operands, EARLY.
            xth0 = cp.tile([P, IBN * HH], f16, tag="xth0", name="xth0")
            nc.sync.dma_start(
                xth0[:].rearrange("p (b c) -> p b c", b=IBN),
                xT[:, 0:HH].rearrange("(b p) c -> p b c", p=P))
            xth0s = [xth0[:, i * HH:(i + 1) * HH] for i in range(IBN)]
            xth1t = cp.tile([P, 2 * HH], f16, tag="xth1t", name="xth1t")
            nc.gpsimd.dma_start(
                xth1t[:].rearrange("p (b c) -> p b c", b=2),
                xT[2 * P:, HH:HF].rearrange("(b p) c -> p b c", p=P))
            btrPr = cp.tile([P, 3 * IN], f16, tag="btrPr", name="btrPr")
            nc.gpsimd.dma_start(btrPr[:], btrP[:, IN:])
            btiPr = cp.tile([P, 3 * IN], f16, tag="btiPr", name="btiPr")
            nc.gpsimd.dma_start(btiPr[:], btiP[:, IN:])
            xw_t = load_consolidated(xw, nc.gpsimd, "xw")
            vre_t = load_consolidated(vre, nc.gpsimd, "vre")
            vim_t = load_consolidated(vim, nc.gpsimd, "vim")

            # sync (continued): trig (twist pacing), small consts, late weights
            cos_t = [None] * NB
            sin_t = [None] * NB
            def load_trig(nb):
                c = cp.tile([P, HF], f16, tag=f"cos{nb}", name=f"cos{nb}")
                s = cp.tile([P, HF], f16, tag=f"sin{nb}", name=f"sin{nb}")
                nc.sync.dma_start(c[:], cosj[nb * P:(nb + 1) * P, :])
                nc.sync.dma_start(s[:], sinj[nb * P:(nb + 1) * P, :])
                cos_t[nb], sin_t[nb] = c, s
            load_trig(0)
            load_trig(1)
            eye_t = cp.tile([P, P], f16, tag="eye", name="eye")
            nc.sync.dma_start(eye_t[:], eye[:, :])
            rb_t = row_tiles(rb, nc.sync, "rb")
            rot_t = row_tiles(rot, nc.sync, "rot")
            load_trig(2)
            load_trig(3)
            dtw_t = load_consolidated(dtw, nc.sync, "dtw")
            ctr_t = load_consolidated(ctr, nc.sync, "ctr")
            ctin_t = load_consolidated(ctin, nc.sync, "ctin")

            # scalar: only the pre-first-evac loads
            def load_blocked(dram, eng, pfx, nb):
                t = cp.tile([P, IN], f16, tag=f"{pfx}{nb}", name=f"{pfx}{nb}")
                eng.dma_start(t[:], dram[:, nb * IN:(nb + 1) * IN])
                return t
            btrP_t = [load_blocked(btrP, nc.scalar, "btrP", 0)]
            btiP_t = [load_blocked(btiP, nc.scalar, "btiP", 0)]
            xh1a = [cp.tile([P, HH], f16, tag=f"xh1a{i}", name=f"xh1a{i}")
                    for i in range(2)]
            nc.scalar.dma_start(xh1a[0][:], xT[0:P, HH:HF])
            nc.scalar.dma_start(xh1a[1][:], xT[P:2 * P, HH:HF])
            btrP_t += [btrPr[:, (nb - 1) * IN:nb * IN] for nb in range(1, NB)]
            btiP_t += [btiPr[:, (nb - 1) * IN:nb * IN] for nb in range(1, NB)]

            def xv(ib, h):
                """x^T [P, HH] view for input block ib, half h."""
                if h == 0:
                    return xth0s[ib]
                if ib < 2:
                    return xh1a[ib][:]
                return xth1t[:, (ib - 2) * HH:(ib - 1) * HH]

            # ---------------- PE building blocks ----------------
            bus = {}

            def input_gemm(nb):
                for pi, wt in enumerate((btrP_t[nb], btiP_t[nb])):
                    bu = wkp.tile([P, HF], f16, tag=f"bu{pi}", bufs=2,
                                  name=f"bu{pi}_{nb}")
                    ps = [bp.tile([P, HH], f32, tag="ps", name=f"bups{h}")
                          for h in range(2)]
                    for ib in range(IBN):
                        for h in range(2):
                            nc.tensor.matmul(
                                ps[h][:], wt[:, ib * P:(ib + 1) * P], xv(ib, h),
                                start=(ib == 0), stop=(ib == IBN - 1))
                    for h in range(2):
                        nc.scalar.copy(bu[:, h * HH:(h + 1) * HH], ps[h][:])
                    bus[(nb, pi)] = bu

            # W-GEMM (flipped): w[i-block, n] = sum_s xw[s, i] * V[s, n]
            # planes: 0 = wre, 1 = wim, 2 = -wim (negated evac copy)
            wts = {}

            def w_gemm(pi):
                vt = (vre_t, vim_t)[pi]
                for ib in range(IBN):
                    ps = bp.tile([P, IN], f32, tag="ps", name="wps")
                    for sb in range(WB):
                        nc.tensor.matmul(
                            ps[:], xw_t[sb][:, ib * P:(ib + 1) * P], vt[sb][:],
                            start=(sb == 0), stop=(sb == WB - 1))
                    w = wkp.tile([P, IN], f16, tag=f"w{pi}{ib}", name=f"w{pi}{ib}")
                    nc.scalar.copy(w[:], ps[:])
                    wts[(pi, ib)] = w
                    if pi == 1:
                        wn = wkp.tile([P, IN], f16, tag=f"w2{ib}", name=f"w2{ib}")
                        nc.scalar.mul(wn[:], ps[:], -1.0)
                        wts[(2, ib)] = wn

            inits = {}

            def diag_carry(nb):
                """a_re/a_im via diag-block MMs + masked STT reduce + rotate."""
                nsl = slice(nb * P, (nb + 1) * P)
                bsl = lambda t, ib: t[:, ib * P:(ib + 1) * P]
                aps = {}
                for name in ("are", "aim"):
                    aps[name] = bp.tile([P, HH], f32, tag="ps", name=name)
                # weight-stationary: btrP feeds both planes; a_re's minus sign
                # rides the negated wim copy (plane 2)
                for ib in range(IBN):
                    nc.tensor.matmul(aps["are"][:, 0:P], bsl(btrP_t[nb], ib),
                                     wts[(0, ib)][:, nsl],
                                     start=(ib == 0), stop=False,
                                     skip_group_check=True)
                    nc.tensor.matmul(aps["aim"][:, 0:P], bsl(btrP_t[nb], ib),
                                     wts[(1, ib)][:, nsl],
                                     start=(ib == 0), stop=False,
                                     skip_group_check=True)
                for ib in range(IBN):
                    nc.tensor.matmul(aps["are"][:, 0:P], bsl(btiP_t[nb], ib),
                                     wts[(2, ib)][:, nsl],
                                     start=False, stop=(ib == IBN - 1),
                                     skip_group_check=True)
                    nc.tensor.matmul(aps["aim"][:, 0:P], bsl(btiP_t[nb], ib),
                                     wts[(0, ib)][:, nsl],
                                     start=False, stop=(ib == IBN - 1),
                                     skip_group_check=True)
                a_re = cp.tile([P, 1], f32, tag=f"are{nb}", name=f"are{nb}")
                a_im = cp.tile([P, 1], f32, tag=f"aim{nb}", name=f"aim{nb}")
                dre = wkp.tile([P, P], f16, tag="dre", name="dre")
                dim = wkp.tile([P, P], f16, tag="dim", name="dim")
                nc.vector.scalar_tensor_tensor(
                    dre[:], aps["are"][:, 0:P], 1.0, eye_t[:], AOP.bypass,
                    AOP.mult, accum_out=a_re[:])
                nc.vector.scalar_tensor_tensor(
                    dim[:], aps["aim"][:, 0:P], 1.0, eye_t[:], AOP.bypass,
                    AOP.mult, accum_out=a_im[:])
                # init = e^{i theta} * a
                i_re = cp.tile([P, 1], f32, tag=f"ire{nb}", name=f"ire{nb}")
                i_im = cp.tile([P, 1], f32, tag=f"iim{nb}", name=f"iim{nb}")
                u_re = cp.tile([P, 1], f32, tag=f"ure{nb}", name=f"ure{nb}")
                u_im = cp.tile([P, 1], f32, tag=f"uim{nb}", name=f"uim{nb}")
                nc.scalar.mul(u_re[:], a_re[:], rot_t[nb][:, 0:1])
                nc.vector.scalar_tensor_tensor(
                    i_re[:], a_im[:], rot_t[nb][:, 1:2], u_re[:], AOP.mult, AOP.add)
                nc.scalar.mul(u_im[:], a_im[:], rot_t[nb][:, 0:1])
                nc.vector.scalar_tensor_tensor(
                    i_im[:], a_re[:], rot_t[nb][:, 2:3], u_im[:], AOP.mult, AOP.add)
                inits[nb] = (i_re, i_im)

            # ---------------- DVE building blocks ----------------
            es = {}

            def twist(nb, h):
                hs = slice(h * HH, (h + 1) * HH)
                ct, st = cos_t[nb], sin_t[nb]
                br, bi = bus[(nb, 0)], bus[(nb, 1)]
                if h == 0:
                    e_re = wkp.tile([P, HF], f16, tag="er", bufs=4, name=f"er{nb}")
                    e_im = wkp.tile([P, HF], f16, tag="ei", bufs=4, name=f"ei{nb}")
                    es[nb] = (e_re, e_im)
                e_re, e_im = es[nb]
                p1 = wkp.tile([P, HH], f16, tag="p1", bufs=2, name="p1")
                p2 = wkp.tile([P, HH], f16, tag="p2", bufs=2, name="p2")
                nc.vector.tensor_tensor(p1[:], ct[:, hs], br[:, hs], AOP.mult)
                nc.vector.tensor_tensor(p2[:], st[:, hs], bi[:, hs], AOP.mult)
                nc.vector.tensor_tensor(e_re[:, hs], p1[:], p2[:], AOP.add)
                nc.vector.tensor_tensor(p1[:], ct[:, hs], bi[:, hs], AOP.mult)
                nc.vector.tensor_tensor(p2[:], st[:, hs], br[:, hs], AOP.mult)
                nc.vector.tensor_tensor(e_im[:, hs], p1[:], p2[:], AOP.subtract)

            ss = {}

            def scan(nb, h):
                i_re, i_im = inits[nb]
                e_re, e_im = es[nb]
                if h == 0:
                    s_re = wkp.tile([P, HF], f16, tag="sr", bufs=4, name=f"sr{nb}")
                    s_im = wkp.tile([P, HF], f16, tag="si", bufs=4, name=f"si{nb}")
                    ss[nb] = (s_re, s_im)
                s_re, s_im = ss[nb]
                hs = slice(h * HH, (h + 1) * HH)
                ir = i_re[:, 0:1] if h == 0 else s_re[:, HH - 1:HH]
                ii = i_im[:, 0:1] if h == 0 else s_im[:, HH - 1:HH]
                rbb = rb_t[nb][:, 0:1].broadcast_to((P, HH))
                nc.vector.tensor_tensor_scan(
                    s_re[:, hs], rbb, e_re[:, hs], ir, AOP.mult, AOP.add)
                nc.vector.tensor_tensor_scan(
                    s_im[:, hs], rbb, e_im[:, hs], ii, AOP.mult, AOP.add)

            hhs = {}

            def untwist(nb, h):
                hs = slice(h * HH, (h + 1) * HH)
                s_re, s_im = ss[nb]
                ct, st = cos_t[nb], sin_t[nb]
                if h == 0:
                    hhr = wkp.tile([P, HF], f16, tag="hhr", bufs=4, name=f"hhr{nb}")
                    hhi = wkp.tile([P, HF], f16, tag="hhi", bufs=4, name=f"hhi{nb}")
                    hhs[nb] = (hhr, hhi)
                hhr, hhi = hhs[nb]
                q1 = wkp.tile([P, HH], f16, tag="q1", bufs=2, name="q1")
                q2 = wkp.tile([P, HH], f16, tag="q2", bufs=2, name="q2")
                q3 = wkp.tile([P, HH], f16, tag="q3", bufs=2, name="q3")
                q4 = wkp.tile([P, HH], f16, tag="q4", bufs=2, name="q4")
                nc.vector.tensor_tensor(q1[:], ct[:, hs], s_re[:, hs], AOP.mult)
                nc.vector.tensor_tensor(q2[:], st[:, hs], s_im[:, hs], AOP.mult)
                nc.vector.tensor_tensor(hhr[:, hs], q1[:], q2[:], AOP.subtract)
                nc.vector.tensor_tensor(q3[:], ct[:, hs], s_im[:, hs], AOP.mult)
                nc.vector.tensor_tensor(q4[:], st[:, hs], s_re[:, hs], AOP.mult)
                nc.vector.tensor_tensor(hhi[:, hs], q3[:], q4[:], AOP.add)

            # ---------------- emission ----------------
            # PE: in0-2, W, diag0 early (carry ready as twists end), in3 +
            # remaining diags after.  DVE: twists interleaved so a late bu3
            # never blocks ready scans; h1-untwists deferred so the h0 output
            # groups close (and store) an untwist-phase earlier.
            input_gemm(0)
            twist(0, 0)
            twist(0, 1)
            input_gemm(1)
            twist(1, 0)
            twist(1, 1)
            input_gemm(2)
            twist(2, 0)
            twist(2, 1)
            w_gemm(0)
            w_gemm(1)
            diag_carry(0)
            input_gemm(3)
            scan(0, 0)
            scan(0, 1)
            untwist(0, 0)
            twist(3, 0)
            twist(3, 1)
            diag_carry(1)
            scan(1, 0)
            scan(1, 1)
            untwist(1, 0)
            diag_carry(2)
            diag_carry(3)

            # output groups + D@x fill
            groups = {}
            for h in range(2):
                for ob in range(OBN):
                    gi = h * OBN + ob
                    pool = yp if gi < 5 else bp
                    tag = "yps" if gi < 5 else "ps"
                    groups[(h, ob)] = pool.tile([P, HH], f32, tag=tag, name="yps")
            for ob in range(OBN):
                osl = slice(ob * P, (ob + 1) * P)
                for ib in range(IBN):
                    for h in range(2):
                        nc.tensor.matmul(
                            groups[(h, ob)][:], dtw_t[ib][:, osl], xv(ib, h),
                            start=(ib == 0), stop=False)

            def store_group(h, ob):
                osl = slice(ob * P, (ob + 1) * P)
                hsl = slice(h * HH, (h + 1) * HH)
                ysb = wkp.tile([P, HH], f16, tag="ysb", bufs=4, name="ysb")
                nc.scalar.copy(ysb[:], groups[(h, ob)][:])
                eng = nc.sync if ob % 2 == 0 else nc.gpsimd
                eng.dma_start(yT[osl, hsl], ysb[:])

            def c_mms(nb, h):
                hhr, hhi = hhs[nb]
                last = nb == NB - 1
                for ob in range(OBN):
                    osl = slice(ob * P, (ob + 1) * P)
                    for wi, (wt, m) in enumerate(
                            ((ctr_t[nb][:, osl], hhr), (ctin_t[nb][:, osl], hhi))):
                        nc.tensor.matmul(
                            groups[(h, ob)][:], wt, m[:, h * HH:(h + 1) * HH],
                            start=False, stop=(last and wi == 1))
                    if last:
                        store_group(h, ob)

            scan(2, 0)
            scan(2, 1)
            untwist(2, 0)
            scan(3, 0)
            scan(3, 1)
            untwist(3, 0)
            c_mms(0, 0)
            c_mms(1, 0)
            c_mms(2, 0)
            c_mms(3, 0)
            untwist(0, 1)
            c_mms(0, 1)
            untwist(1, 1)
            c_mms(1, 1)
            untwist(2, 1)
            c_mms(2, 1)
            untwist(3, 1)
            c_mms(3, 1)

    nc.compile()
    nc.finalize()
    _CACHE["nc"] = nc
    return nc


def _host_prep(x, nu_log, theta_log, gamma_log, B_re, B_im, C_re, C_im, D):
    f64 = np.float64
    nu = np.asarray(nu_log, f64)
    th = np.asarray(theta_log, f64)
    gl = np.asarray(gamma_log, f64)
    r = np.exp(-np.exp(nu))
    theta = np.exp(th)
    gamma = np.exp(gl)

    gbr = gamma[:, None] * np.asarray(B_re, f64)   # [n, i]
    gbi = gamma[:, None] * np.asarray(B_im, f64)

    def blocked(w_ni):
        """[n, i] -> [p, (nb, ib, nl)] with p = i within block."""
        wt = np.ascontiguousarray(w_ni.T)  # [i, n]
        b = wt.reshape(IBN, P, NB, P)      # (ib, p, nb, nl)
        return np.ascontiguousarray(b.transpose(1, 2, 0, 3).reshape(P, NB * IN)
                                    ).astype(np.float16)

    shared = {
        "btrP": blocked(gbr),
        "btiP": blocked(gbi),
        "ctr": np.ascontiguousarray(np.asarray(C_re, f64).T).astype(np.float16),
        "ctin": np.ascontiguousarray((-np.asarray(C_im, f64)).T).astype(np.float16),
        "dtw": np.ascontiguousarray(np.asarray(D, f64).T).astype(np.float16),
        "eye": np.eye(P, dtype=np.float16),
    }
    j = np.arange(HF, dtype=f64)
    ang = theta[:, None] * j[None, :]
    shared["cosj"] = np.cos(ang).astype(np.float16)
    shared["sinj"] = np.sin(ang).astype(np.float16)
    # V = lam^{511-s} over the LAST WCON steps of the first half, [s, n]
    e = (WCON - 1) - np.arange(WCON, dtype=f64)
    mag = np.exp(np.log(r)[:, None] * e[None, :])
    angv = theta[:, None] * e[None, :]
    shared["vre"] = np.ascontiguousarray((mag * np.cos(angv)).T).astype(np.float16)
    shared["vim"] = np.ascontiguousarray((mag * np.sin(angv)).T).astype(np.float16)
    shared["rb"] = np.ascontiguousarray(r[:, None].astype(np.float32))
    shared["rot"] = np.stack(
        [np.cos(theta), -np.sin(theta), np.sin(theta)], axis=1).astype(np.float32)

    x = np.asarray(x, np.float32)
    in_maps = []
    for b in range(BATCH):
        m = dict(shared)
        m["xT"] = np.ascontiguousarray(x[b, HF:].T).astype(np.float16)
        m["xw"] = np.ascontiguousarray(x[b, HF - WCON:HF]).astype(np.float16)
        in_maps.append(m)
    return in_maps


def _run(in_maps, trace=False):
    nc = _build_nc()
    return run_bass_kernel_spmd(nc, in_maps, core_ids=list(range(BATCH)), trace=trace)


def kernel(**inputs):
    in_maps = _host_prep(**inputs)
    res = _run(in_maps, trace=False)
    y = np.stack([np.ascontiguousarray(res.results[b]["yT"].T) for b in range(BATCH)])
    return y.astype(np.float32)


def kernel_traced(**inputs):
    """Like kernel() but returns (y, exec_time_ns). Used by test.py."""
    in_maps = _host_prep(**inputs)
    res = _run(in_maps, trace=True)
    y = np.stack([np.ascontiguousarray(res.results[b]["yT"].T) for b in range(BATCH)])
    return y.astype(np.float32), res.exec_time_ns


# revision 37
# speedup vs baseline: 1.0147x; 1.0147x over previous
"""Trainium2 Bass kernel for CausalDownsamplingLRU.

Algorithm (per core = one batch element; 8 cores, data-parallel over batch):
  With lam = r*e^{i theta} (per state n), h_t = lam*h_{t-1} + Bu_t, and only
  y[:, -DS:] needed:

  1. Input GEMMs (fp16, PE): Bu^T[n,t] = (gamma*B)^T.T @ x^T  (re & im planes)
  2. FIRST half: only h_{1023} (the carry into the output window) is needed,
     truncated to the last WCON=512 steps (error <= 0.99^512 ~ 0.6%):
        w[i,n]  = sum_s x_s[i] V[s,n],  V = lam^{511-s}     (PE, flipped GEMM)
        a[n]    = sum_i (gamma*B)[n,i] (.) w[i,n]           (PE diag-block MMs
                  into PSUM [n_p, n_f]; diagonal extracted by a masked
                  STT-reduce against an identity tile)
  3. SECOND half: phase twist e_j = e^{-i j theta} (.) Bu_{1024+j} decouples
     the complex recurrence into two REAL per-partition scans
        s_j = r*s_{j-1} + e_j   (tensor_tensor_scan, fp32 state),
     with initial s_{-1} = e^{i theta} * h_1023.
  4. Untwist h = e^{+i j theta} (.) s, then output GEMMs:
        y^T = C_re^T.T @ h_re + (-C_im^T).T @ h_im + D^T.T @ x^T

All elementwise work stays on the DVE (GPSIMD compute shares the DVE SBUF
port: measured ~4x mutual slowdown, so it only issues DMAs here).  Weights
are shipped nb-blocked (btrP) so each state-block's pipeline starts as soon
as its own 0.25MB lands; the 2MB trig tables get a dedicated DMA path since
they pace the twists.
"""
import numpy as np

import concourse.bass as bass
import concourse.bacc as bacc
import concourse.mybir as mybir
from concourse.tile import TileContext
from concourse.bass_utils import run_bass_kernel_spmd

BATCH, T, IN, OUT, N = 8, 2048, 512, 512, 512
DS = 1024
P = 128
NB = N // P    # 4 state blocks
IBN = IN // P  # 4 input blocks
OBN = OUT // P # 4 output blocks
HF = 1024      # window length (= DS)
HH = 512       # half length (PSUM bank limit for f32 out)
WCON = 512     # carry W-GEMM contraction (last WCON steps of first half)
WB = WCON // P

f32 = mybir.dt.float32
f16 = mybir.dt.float16
AOP = mybir.AluOpType

_CACHE = {}


def _build_nc():
    if "nc" in _CACHE:
        return _CACHE["nc"]
    nc = bacc.Bacc()
    xT = nc.dram_tensor("xT", [IN, HF], f16, kind="ExternalInput")    # 2nd half, [i, t]
    xw = nc.dram_tensor("xw", [WCON, IN], f16, kind="ExternalInput")  # x[512:1024], [s, i]
    # nb-blocked weights: [p, nb*IN] with cols (nb, ib, n-within-block)
    btrP = nc.dram_tensor("btrP", [P, NB * IN], f16, kind="ExternalInput")
    btiP = nc.dram_tensor("btiP", [P, NB * IN], f16, kind="ExternalInput")
    vre = nc.dram_tensor("vre", [WCON, N], f16, kind="ExternalInput")
    vim = nc.dram_tensor("vim", [WCON, N], f16, kind="ExternalInput")
    cosj = nc.dram_tensor("cosj", [N, HF], f16, kind="ExternalInput")
    sinj = nc.dram_tensor("sinj", [N, HF], f16, kind="ExternalInput")
    eye = nc.dram_tensor("eye", [P, P], f16, kind="ExternalInput")
    rb = nc.dram_tensor("rb", [N, 1], f32, kind="ExternalInput")
    # rot columns: 0=cos(theta), 1=-sin(theta), 2=sin(theta)
    rot = nc.dram_tensor("rot", [N, 3], f32, kind="ExternalInput")
    ctr = nc.dram_tensor("ctr", [N, OUT], f16, kind="ExternalInput")
    ctin = nc.dram_tensor("ctin", [N, OUT], f16, kind="ExternalInput")
    dtw = nc.dram_tensor("dtw", [IN, OUT], f16, kind="ExternalInput")
    yT = nc.dram_tensor("yT", [OUT, DS], f16, kind="ExternalOutput")

    with TileContext(nc) as tc:
        with (
            tc.tile_pool(name="const", bufs=1) as cp,
            tc.tile_pool(name="work", bufs=1) as wkp,
            tc.tile_pool(name="ps", bufs=3, space="PSUM") as bp,
            tc.tile_pool(name="yps", bufs=5, space="PSUM") as yp,
        ):
            def row_tiles(dram, eng, pfx, cols=None):
                rows = dram.shape[0]
                cols = cols if cols is not None else dram.shape[1]
                out = []
                for i in range(rows // P):
                    t = cp.tile([P, cols], dram.dtype, tag=f"{pfx}{i}", name=f"{pfx}{i}")
                    eng.dma_start(t[:], dram[i * P:(i + 1) * P, 0:cols])
                    out.append(t)
                return out

            # --- DMA map.  Each issue-queue delivers ~90GB/s SERIALLY; the
            # gpsimd SWDGE descriptor writes contend with the DVE SBUF port
            # (+20% DVE time measured), so ALL gpsimd loads are consolidated
            # and issued up-front to finish before the vector phase; scalar
            # (the evac engine) only carries pre-first-evac loads; sync
            # carries the trig tables (they pace the twists) + late weights.
            def load_consolidated(dram, eng, tagp, cols=None, col0=0):
                """One rearranged DMA for a [R*P, C] dram table -> row tiles."""
                rows = dram.shape[0]
                nb_ = rows // P
                cols = cols if cols is not None else dram.shape[1]
                big = cp.tile([P, nb_ * cols], dram.dtype, tag=tagp, name=tagp)
                eng.dma_start(
                    big[:].rearrange("p (b c) -> p b c", b=nb_),
                    dram[:, col0:col0 + cols].rearrange("(b p) c -> p b c", p=P))
                return [big[:, i * cols:(i + 1) * cols] for i in range(nb_)]

            # gpsimd: everything x + input weights + w-gemm operands, EARLY
            xth0 = cp.tile([P, IBN * HH], f16, tag="xth0", name="xth0")
            nc.gpsimd.dma_start(
                xth0[:].rearrange("p (b c) -> p b c", b=IBN),
                xT[:, 0:HH].rearrange("(b p) c -> p b c", p=P))
            xth0s = [xth0[:, i * HH:(i + 1) * HH] for i in range(IBN)]
            xth1t = cp.tile([P, 2 * HH], f16, tag="xth1t", name="xth1t")
            nc.gpsimd.dma_start(
                xth1t[:].rearrange("p (b c) -> p b c", b=2),
                xT[2 * P:, HH:HF].rearrange("(b p) c -> p b c", p=P))
            btrPr = cp.tile([P, 3 * IN], f16, tag="btrPr", name="btrPr")
            nc.gpsimd.dma_start(btrPr[:], btrP[:, IN:])
            btiPr = cp.tile([P, 3 * IN], f16, tag="btiPr", name="btiPr")
            nc.gpsimd.dma_start(btiPr[:], btiP[:, IN:])
            xw_t = load_consolidated(xw, nc.gpsimd, "xw")
            vre_t = load_consolidated(vre, nc.gpsimd, "vre")
            vim_t = load_consolidated(vim, nc.gpsimd, "vim")

            # sync: eye, trig (twist pacing), small consts, late out-weights
            eye_t = cp.tile([P, P], f16, tag="eye", name="eye")
            nc.sync.dma_start(eye_t[:], eye[:, :])
            cos_t = [None] * NB
            sin_t = [None] * NB
            def load_trig(nb):
                c = cp.tile([P, HF], f16, tag=f"cos{nb}", name=f"cos{nb}")
                s = cp.tile([P, HF], f16, tag=f"sin{nb}", name=f"sin{nb}")
                nc.sync.dma_start(c[:], cosj[nb * P:(nb + 1) * P, :])
                nc.sync.dma_start(s[:], sinj[nb * P:(nb + 1) * P, :])
                cos_t[nb], sin_t[nb] = c, s
            load_trig(0)
            load_trig(1)
            rb_t = row_tiles(rb, nc.sync, "rb")
            rot_t = row_tiles(rot, nc.sync, "rot")
            load_trig(2)
            load_trig(3)
            dtw_t = load_consolidated(dtw, nc.sync, "dtw")
            ctr_t = load_consolidated(ctr, nc.sync, "ctr")
            ctin_t = load_consolidated(ctin, nc.sync, "ctin")

            # scalar: only the pre-first-evac loads
            def load_blocked(dram, eng, pfx, nb):
                t = cp.tile([P, IN], f16, tag=f"{pfx}{nb}", name=f"{pfx}{nb}")
                eng.dma_start(t[:], dram[:, nb * IN:(nb + 1) * IN])
                return t
            btrP_t = [load_blocked(btrP, nc.scalar, "btrP", 0)]
            btiP_t = [load_blocked(btiP, nc.scalar, "btiP", 0)]
            xh1a = [cp.tile([P, HH], f16, tag=f"xh1a{i}", name=f"xh1a{i}")
                    for i in range(2)]
            nc.scalar.dma_start(xh1a[0][:], xT[0:P, HH:HF])
            nc.scalar.dma_start(xh1a[1][:], xT[P:2 * P, HH:HF])
            btrP_t += [btrPr[:, (nb - 1) * IN:nb * IN] for nb in range(1, NB)]
            btiP_t += [btiPr[:, (nb - 1) * IN:nb * IN] for nb in range(1, NB)]

            def xv(ib, h):
                """x^T [P, HH] view for input block ib, half h."""
                if h == 0:
                    return xth0s[ib]
                if ib < 2:
                    return xh1a[ib][:]
                return xth1t[:, (ib - 2) * HH:(ib - 1) * HH]

            # ---------------- PE building blocks ----------------
            bus = {}

            def input_gemm(nb):
                for pi, wt in enumerate((btrP_t[nb], btiP_t[nb])):
                    bu = wkp.tile([P, HF], f16, tag=f"bu{pi}", bufs=2,
                                  name=f"bu{pi}_{nb}")
                    ps = [bp.tile([P, HH], f32, tag="ps", name=f"bups{h}")
                          for h in range(2)]
                    for ib in range(IBN):
                        for h in range(2):
                            nc.tensor.matmul(
                                ps[h][:], wt[:, ib * P:(ib + 1) * P], xv(ib, h),
                                start=(ib == 0), stop=(ib == IBN - 1))
                    for h in range(2):
                        nc.scalar.copy(bu[:, h * HH:(h + 1) * HH], ps[h][:])
                    bus[(nb, pi)] = bu

            # W-GEMM (flipped): w[i-block, n] = sum_s xw[s, i] * V[s, n]
            # planes: 0 = wre, 1 = wim, 2 = -wim (negated evac copy)
            wts = {}

            def w_gemm(pi):
                vt = (vre_t, vim_t)[pi]
                for ib in range(IBN):
                    ps = bp.tile([P, IN], f32, tag="ps", name="wps")
                    for sb in range(WB):
                        nc.tensor.matmul(
                            ps[:], xw_t[sb][:, ib * P:(ib + 1) * P], vt[sb][:],
                            start=(sb == 0), stop=(sb == WB - 1))
                    w = wkp.tile([P, IN], f16, tag=f"w{pi}{ib}", name=f"w{pi}{ib}")
                    nc.scalar.copy(w[:], ps[:])
                    wts[(pi, ib)] = w
                    if pi == 1:
                        wn = wkp.tile([P, IN], f16, tag=f"w2{ib}", name=f"w2{ib}")
                        nc.scalar.mul(wn[:], ps[:], -1.0)
                        wts[(2, ib)] = wn

            inits = {}

            def diag_carry(nb):
                """a_re/a_im via diag-block MMs + masked STT reduce + rotate."""
                nsl = slice(nb * P, (nb + 1) * P)
                bsl = lambda t, ib: t[:, ib * P:(ib + 1) * P]
                aps = {}
                for name in ("are", "aim"):
                    aps[name] = bp.tile([P, HH], f32, tag="ps", name=name)
                # weight-stationary: btrP feeds both planes; a_re's minus sign
                # rides the negated wim copy (plane 2)
                for ib in range(IBN):
                    nc.tensor.matmul(aps["are"][:, 0:P], bsl(btrP_t[nb], ib),
                                     wts[(0, ib)][:, nsl],
                                     start=(ib == 0), stop=False,
                                     skip_group_check=True)
                    nc.tensor.matmul(aps["aim"][:, 0:P], bsl(btrP_t[nb], ib),
                                     wts[(1, ib)][:, nsl],
                                     start=(ib == 0), stop=False,
                                     skip_group_check=True)
                for ib in range(IBN):
                    nc.tensor.matmul(aps["are"][:, 0:P], bsl(btiP_t[nb], ib),
                                     wts[(2, ib)][:, nsl],
                                     start=False, stop=(ib == IBN - 1),
                                     skip_group_check=True)
                    nc.tensor.matmul(aps["aim"][:, 0:P], bsl(btiP_t[nb], ib),
                                     wts[(0, ib)][:, nsl],
                                     start=False, stop=(ib == IBN - 1),
                                     skip_group_check=True)
                a_re = cp.tile([P, 1], f32, tag=f"are{nb}", name=f"are{nb}")
                a_im = cp.tile([P, 1], f32, tag=f"aim{nb}", name=f"aim{nb}")
                dre = wkp.tile([P, P], f16, tag="dre", name="dre")
                dim = wkp.tile([P, P], f16, tag="dim", name="dim")
                nc.vector.scalar_tensor_tensor(
                    dre[:], aps["are"][:, 0:P], 1.0, eye_t[:], AOP.bypass,
                    AOP.mult, accum_out=a_re[:])
                nc.vector.scalar_tensor_tensor(
                    dim[:], aps["aim"][:, 0:P], 1.0, eye_t[:], AOP.bypass,
                    AOP.mult, accum_out=a_im[:])
                # init = e^{i theta} * a
                i_re = cp.tile([P, 1], f32, tag=f"ire{nb}", name=f"ire{nb}")
                i_im = cp.tile([P, 1], f32, tag=f"iim{nb}", name=f"iim{nb}")
                u_re = cp.tile([P, 1], f32, tag=f"ure{nb}", name=f"ure{nb}")
                u_im = cp.tile([P, 1], f32, tag=f"uim{nb}", name=f"uim{nb}")
                nc.scalar.mul(u_re[:], a_re[:], rot_t[nb][:, 0:1])
                nc.vector.scalar_tensor_tensor(
                    i_re[:], a_im[:], rot_t[nb][:, 1:2], u_re[:], AOP.mult, AOP.add)
                nc.scalar.mul(u_im[:], a_im[:], rot_t[nb][:, 0:1])
                nc.vector.scalar_tensor_tensor(
                    i_im[:], a_re[:], rot_t[nb][:, 2:3], u_im[:], AOP.mult, AOP.add)
                inits[nb] = (i_re, i_im)

            # ---------------- DVE building blocks ----------------
            es = {}

            def twist(nb, h):
                hs = slice(h * HH, (h + 1) * HH)
                ct, st = cos_t[nb], sin_t[nb]
                br, bi = bus[(nb, 0)], bus[(nb, 1)]
                if h == 0:
                    e_re = wkp.tile([P, HF], f16, tag="er", bufs=4, name=f"er{nb}")
                    e_im = wkp.tile([P, HF], f16, tag="ei", bufs=4, name=f"ei{nb}")
                    es[nb] = (e_re, e_im)
                e_re, e_im = es[nb]
                p1 = wkp.tile([P, HH], f16, tag="p1", bufs=2, name="p1")
                p2 = wkp.tile([P, HH], f16, tag="p2", bufs=2, name="p2")
                nc.vector.tensor_tensor(p1[:], ct[:, hs], br[:, hs], AOP.mult)
                nc.vector.tensor_tensor(p2[:], st[:, hs], bi[:, hs], AOP.mult)
                nc.vector.tensor_tensor(e_re[:, hs], p1[:], p2[:], AOP.add)
                nc.vector.tensor_tensor(p1[:], ct[:, hs], bi[:, hs], AOP.mult)
                nc.vector.tensor_tensor(p2[:], st[:, hs], br[:, hs], AOP.mult)
                nc.vector.tensor_tensor(e_im[:, hs], p1[:], p2[:], AOP.subtract)

            ss = {}

            def scan(nb, h):
                i_re, i_im = inits[nb]
                e_re, e_im = es[nb]
                if h == 0:
                    s_re = wkp.tile([P, HF], f16, tag="sr", bufs=4, name=f"sr{nb}")
                    s_im = wkp.tile([P, HF], f16, tag="si", bufs=4, name=f"si{nb}")
                    ss[nb] = (s_re, s_im)
                s_re, s_im = ss[nb]
                hs = slice(h * HH, (h + 1) * HH)
                ir = i_re[:, 0:1] if h == 0 else s_re[:, HH - 1:HH]
                ii = i_im[:, 0:1] if h == 0 else s_im[:, HH - 1:HH]
                rbb = rb_t[nb][:, 0:1].broadcast_to((P, HH))
                nc.vector.tensor_tensor_scan(
                    s_re[:, hs], rbb, e_re[:, hs], ir, AOP.mult, AOP.add)
                nc.vector.tensor_tensor_scan(
                    s_im[:, hs], rbb, e_im[:, hs], ii, AOP.mult, AOP.add)

            hhs = {}

            def untwist(nb, h):
                hs = slice(h * HH, (h + 1) * HH)
                s_re, s_im = ss[nb]
                ct, st = cos_t[nb], sin_t[nb]
                if h == 0:
                    hhr = wkp.tile([P, HF], f16, tag="hhr", bufs=4, name=f"hhr{nb}")
                    hhi = wkp.tile([P, HF], f16, tag="hhi", bufs=4, name=f"hhi{nb}")
                    hhs[nb] = (hhr, hhi)
                hhr, hhi = hhs[nb]
                q1 = wkp.tile([P, HH], f16, tag="q1", bufs=2, name="q1")
                q2 = wkp.tile([P, HH], f16, tag="q2", bufs=2, name="q2")
                q3 = wkp.tile([P, HH], f16, tag="q3", bufs=2, name="q3")
                q4 = wkp.tile([P, HH], f16, tag="q4", bufs=2, name="q4")
                nc.vector.tensor_tensor(q1[:], ct[:, hs], s_re[:, hs], AOP.mult)
                nc.vector.tensor_tensor(q2[:], st[:, hs], s_im[:, hs], AOP.mult)
                nc.vector.tensor_tensor(hhr[:, hs], q1[:], q2[:], AOP.subtract)
                nc.vector.tensor_tensor(q3[:], ct[:, hs], s_im[:, hs], AOP.mult)
                nc.vector.tensor_tensor(q4[:], st[:, hs], s_re[:, hs], AOP.mult)
                nc.vector.tensor_tensor(hhi[:, hs], q3[:], q4[:], AOP.add)

            # ---------------- emission ----------------
            # PE: in0-2, W, diag0 early (carry ready as twists end), in3 +
            # remaining diags after.  DVE: twists interleaved so a late bu3
            # never blocks ready scans; h1-untwists deferred so the h0 output
            # groups close (and store) an untwist-phase earlier.
            input_gemm(0)
            twist(0, 0)
            twist(0, 1)
            input_gemm(1)
            twist(1, 0)
            twist(1, 1)
            input_gemm(2)
            twist(2, 0)
            twist(2, 1)
            w_gemm(0)
            w_gemm(1)
            diag_carry(0)
            input_gemm(3)
            scan(0, 0)
            scan(0, 1)
            untwist(0, 0)
            twist(3, 0)
            twist(3, 1)
            diag_carry(1)
            scan(1, 0)
            scan(1, 1)
            untwist(1, 0)
            diag_carry(2)
            diag_carry(3)

            # output groups + D@x fill
            groups = {}
            for h in range(2):
                for ob in range(OBN):
                    gi = h * OBN + ob
                    pool = yp if gi < 5 else bp
                    tag = "yps" if gi < 5 else "ps"
                    groups[(h, ob)] = pool.tile([P, HH], f32, tag=tag, name="yps")
            for ob in range(OBN):
                osl = slice(ob * P, (ob + 1) * P)
                for ib in range(IBN):
                    for h in range(2):
                        nc.tensor.matmul(
                            groups[(h, ob)][:], dtw_t[ib][:, osl], xv(ib, h),
                            start=(ib == 0), stop=False)

            def store_group(h, ob):
                osl = slice(ob * P, (ob + 1) * P)
                hsl = slice(h * HH, (h + 1) * HH)
                ysb = wkp.tile([P, HH], f16, tag="ysb", bufs=4, name="ysb")
                nc.scalar.copy(ysb[:], groups[(h, ob)][:])
                eng = nc.sync if ob % 2 == 0 else nc.gpsimd
                eng.dma_start(yT[osl, hsl], ysb[:])

            def c_mms(nb, h):
                hhr, hhi = hhs[nb]
                last = nb == NB - 1
                for ob in range(OBN):
                    osl = slice(ob * P, (ob + 1) * P)
                    for wi, (wt, m) in enumerate(
                            ((ctr_t[nb][:, osl], hhr), (ctin_t[nb][:, osl], hhi))):
                        nc.tensor.matmul(
                            groups[(h, ob)][:], wt, m[:, h * HH:(h + 1) * HH],
                            start=False, stop=(last and wi == 1))
                    if last:
                        store_group(h, ob)

            scan(2, 0)
            scan(2, 1)
            untwist(2, 0)
            scan(3, 0)
            scan(3, 1)
            untwist(3, 0)
            c_mms(0, 0)
            c_mms(1, 0)
            c_mms(2, 0)
            c_mms(3, 0)
            untwist(0, 1)
            c_mms(0, 1)
            untwist(1, 1)
            c_mms(1, 1)
            untwist(2, 1)
            c_mms(2, 1)
            untwist(3, 1)
            c_mms(3, 1)

    nc.compile()
    nc.finalize()
    _CACHE["nc"] = nc
    return nc


def _host_prep(x, nu_log, theta_log, gamma_log, B_re, B_im, C_re, C_im, D):
    f64 = np.float64
    nu = np.asarray(nu_log, f64)
    th = np.asarray(theta_log, f64)
    gl = np.asarray(gamma_log, f64)
    r = np.exp(-np.exp(nu))
    theta = np.exp(th)
    gamma = np.exp(gl)

    gbr = gamma[:, None] * np.asarray(B_re, f64)   # [n, i]
    gbi = gamma[:, None] * np.asarray(B_im, f64)

    def blocked(w_ni):
        """[n, i] -> [p, (nb, ib, nl)] with p = i within block."""
        wt = np.ascontiguousarray(w_ni.T)  # [i, n]
        b = wt.reshape(IBN, P, NB, P)      # (ib, p, nb, nl)
        return np.ascontiguousarray(b.transpose(1, 2, 0, 3).reshape(P, NB * IN)
                                    ).astype(np.float16)

    shared = {
        "btrP": blocked(gbr),
        "btiP": blocked(gbi),
        "ctr": np.ascontiguousarray(np.asarray(C_re, f64).T).astype(np.float16),
        "ctin": np.ascontiguousarray((-np.asarray(C_im, f64)).T).astype(np.float16),
        "dtw": np.ascontiguousarray(np.asarray(D, f64).T).astype(np.float16),
        "eye": np.eye(P, dtype=np.float16),
    }
    j = np.arange(HF, dtype=f64)
    ang = theta[:, None] * j[None, :]
    shared["cosj"] = np.cos(ang).astype(np.float16)
    shared["sinj"] = np.sin(ang).astype(np.float16)
    # V = lam^{511-s} over the LAST WCON steps of the first half, [s, n]
    e = (WCON - 1) - np.arange(WCON, dtype=f64)
    mag = np.exp(np.log(r)[:, None] * e[None, :])
    angv = theta[:, None] * e[None, :]
    shared["vre"] = np.ascontiguousarray((mag * np.cos(angv)).T).astype(np.float16)
    shared["vim"] = np.ascontiguousarray((mag * np.sin(angv)).T).astype(np.float16)
    shared["rb"] = np.ascontiguousarray(r[:, None].astype(np.float32))
    shared["rot"] = np.stack(
        [np.cos(theta), -np.sin(theta), np.sin(theta)], axis=1).astype(np.float32)

    x = np.asarray(x, np.float32)
    in_maps = []
    for b in range(BATCH):
        m = dict(shared)
        m["xT"] = np.ascontiguousarray(x[b, HF:].T).astype(np.float16)
        m["xw"] = np.ascontiguousarray(x[b, HF - WCON:HF]).astype(np.float16)
        in_maps.append(m)
    return in_maps


def _run(in_maps, trace=False):
    nc = _build_nc()
    return run_bass_kernel_spmd(nc, in_maps, core_ids=list(range(BATCH)), trace=trace)


def kernel(**inputs):
    in_maps = _host_prep(**inputs)
    res = _run(in_maps, trace=False)
    y = np.stack([np.ascontiguousarray(res.results[b]["yT"].T) for b in range(BATCH)])
    return y.astype(np.float32)


def kernel_traced(**inputs):
    """Like kernel() but returns (y, exec_time_ns). Used by test.py."""
    in_maps = _host_prep(**inputs)
    res = _run(in_maps, trace=True)
    y = np.stack([np.ascontiguousarray(res.results[b]["yT"].T) for b in range(BATCH)])
    return y.astype(np.float32), res.exec_time_ns


# revision 39
# speedup vs baseline: 1.0210x; 1.0062x over previous
"""Trainium2 Bass kernel for CausalDownsamplingLRU.

Algorithm (per core = one batch element; 8 cores, data-parallel over batch):
  With lam = r*e^{i theta} (per state n), h_t = lam*h_{t-1} + Bu_t, and only
  y[:, -DS:] needed:

  1. Input GEMMs (fp16, PE): Bu^T[n,t] = (gamma*B)^T.T @ x^T  (re & im planes)
  2. FIRST half: only h_{1023} (the carry into the output window) is needed,
     truncated to the last WCON=512 steps (error <= 0.99^512 ~ 0.6%):
        w[i,n]  = sum_s x_s[i] V[s,n],  V = lam^{511-s}     (PE, flipped GEMM)
        a[n]    = sum_i (gamma*B)[n,i] (.) w[i,n]           (PE diag-block MMs
                  into PSUM [n_p, n_f]; diagonal extracted by a masked
                  STT-reduce against an identity tile)
  3. SECOND half: phase twist e_j = e^{-i j theta} (.) Bu_{1024+j} decouples
     the complex recurrence into two REAL per-partition scans
        s_j = r*s_{j-1} + e_j   (tensor_tensor_scan, fp32 state),
     with initial s_{-1} = e^{i theta} * h_1023.
  4. Untwist h = e^{+i j theta} (.) s, then output GEMMs:
        y^T = C_re^T.T @ h_re + (-C_im^T).T @ h_im + D^T.T @ x^T

All elementwise work stays on the DVE (GPSIMD compute shares the DVE SBUF
port: measured ~4x mutual slowdown, so it only issues DMAs here).  Weights
are shipped nb-blocked (btrP) so each state-block's pipeline starts as soon
as its own 0.25MB lands; the 2MB trig tables get a dedicated DMA path since
they pace the twists.
"""
import numpy as np

import concourse.bass as bass
import concourse.bacc as bacc
import concourse.mybir as mybir
from concourse.tile import TileContext
from concourse.bass_utils import run_bass_kernel_spmd

BATCH, T, IN, OUT, N = 8, 2048, 512, 512, 512
DS = 1024
P = 128
NB = N // P    # 4 state blocks
IBN = IN // P  # 4 input blocks
OBN = OUT // P # 4 output blocks
HF = 1024      # window length (= DS)
HH = 512       # half length (PSUM bank limit for f32 out)
WCON = 512     # carry W-GEMM contraction (last WCON steps of first half)
WB = WCON // P

f32 = mybir.dt.float32
f16 = mybir.dt.float16
AOP = mybir.AluOpType

_CACHE = {}


def _build_nc():
    if "nc" in _CACHE:
        return _CACHE["nc"]
    nc = bacc.Bacc()
    xT = nc.dram_tensor("xT", [IN, HF], f16, kind="ExternalInput")    # 2nd half, [i, t]
    xw = nc.dram_tensor("xw", [WCON, IN], f16, kind="ExternalInput")  # x[512:1024], [s, i]
    # nb-blocked weights: [p, nb*IN] with cols (nb, ib, n-within-block)
    btrP = nc.dram_tensor("btrP", [P, NB * IN], f16, kind="ExternalInput")
    btiP = nc.dram_tensor("btiP", [P, NB * IN], f16, kind="ExternalInput")
    vre = nc.dram_tensor("vre", [WCON, N], f16, kind="ExternalInput")
    vim = nc.dram_tensor("vim", [WCON, N], f16, kind="ExternalInput")
    cosj = nc.dram_tensor("cosj", [N, HF], f16, kind="ExternalInput")
    sinj = nc.dram_tensor("sinj", [N, HF], f16, kind="ExternalInput")
    eye = nc.dram_tensor("eye", [P, P], f16, kind="ExternalInput")
    rb = nc.dram_tensor("rb", [N, 1], f32, kind="ExternalInput")
    # rot columns: 0=cos(theta), 1=-sin(theta), 2=sin(theta)
    rot = nc.dram_tensor("rot", [N, 3], f32, kind="ExternalInput")
    ctr = nc.dram_tensor("ctr", [N, OUT], f16, kind="ExternalInput")
    ctin = nc.dram_tensor("ctin", [N, OUT], f16, kind="ExternalInput")
    dtw = nc.dram_tensor("dtw", [IN, OUT], f16, kind="ExternalInput")
    yT = nc.dram_tensor("yT", [OUT, DS], f16, kind="ExternalOutput")

    with TileContext(nc) as tc:
        with (
            tc.tile_pool(name="const", bufs=1) as cp,
            tc.tile_pool(name="work", bufs=1) as wkp,
            tc.tile_pool(name="ps", bufs=3, space="PSUM") as bp,
            tc.tile_pool(name="yps", bufs=5, space="PSUM") as yp,
        ):
            def row_tiles(dram, eng, pfx, cols=None):
                rows = dram.shape[0]
                cols = cols if cols is not None else dram.shape[1]
                out = []
                for i in range(rows // P):
                    t = cp.tile([P, cols], dram.dtype, tag=f"{pfx}{i}", name=f"{pfx}{i}")
                    eng.dma_start(t[:], dram[i * P:(i + 1) * P, 0:cols])
                    out.append(t)
                return out

            # --- DMA map.  Each issue-queue delivers ~90GB/s SERIALLY; the
            # gpsimd SWDGE descriptor writes contend with the DVE SBUF port
            # (+20% DVE time measured), so ALL gpsimd loads are consolidated
            # and issued up-front to finish before the vector phase; scalar
            # (the evac engine) only carries pre-first-evac loads; sync
            # carries the trig tables (they pace the twists) + late weights.
            def load_consolidated(dram, eng, tagp, cols=None, col0=0):
                """One rearranged DMA for a [R*P, C] dram table -> row tiles."""
                rows = dram.shape[0]
                nb_ = rows // P
                cols = cols if cols is not None else dram.shape[1]
                big = cp.tile([P, nb_ * cols], dram.dtype, tag=tagp, name=tagp)
                eng.dma_start(
                    big[:].rearrange("p (b c) -> p b c", b=nb_),
                    dram[:, col0:col0 + cols].rearrange("(b p) c -> p b c", p=P))
                return [big[:, i * cols:(i + 1) * cols] for i in range(nb_)]

            # gpsimd: everything x + input weights + w-gemm operands, EARLY
            xth0 = cp.tile([P, IBN * HH], f16, tag="xth0", name="xth0")
            nc.gpsimd.dma_start(
                xth0[:].rearrange("p (b c) -> p b c", b=IBN),
                xT[:, 0:HH].rearrange("(b p) c -> p b c", p=P))
            xth0s = [xth0[:, i * HH:(i + 1) * HH] for i in range(IBN)]
            xth1t = cp.tile([P, 2 * HH], f16, tag="xth1t", name="xth1t")
            nc.gpsimd.dma_start(
                xth1t[:].rearrange("p (b c) -> p b c", b=2),
                xT[2 * P:, HH:HF].rearrange("(b p) c -> p b c", p=P))
            btrPr = cp.tile([P, 3 * IN], f16, tag="btrPr", name="btrPr")
            nc.gpsimd.dma_start(btrPr[:], btrP[:, IN:])
            btiPr = cp.tile([P, 3 * IN], f16, tag="btiPr", name="btiPr")
            nc.gpsimd.dma_start(btiPr[:], btiP[:, IN:])
            xw_t = load_consolidated(xw, nc.gpsimd, "xw")
            vre_t = load_consolidated(vre, nc.gpsimd, "vre")
            vim_t = load_consolidated(vim, nc.gpsimd, "vim")

            # sync: eye, trig (twist pacing), small consts, late out-weights
            eye_t = cp.tile([P, P], f16, tag="eye", name="eye")
            nc.sync.dma_start(eye_t[:], eye[:, :])
            cos_t = [None] * NB
            sin_t = [None] * NB
            def load_trig(nb):
                c = cp.tile([P, HF], f16, tag=f"cos{nb}", name=f"cos{nb}")
                s = cp.tile([P, HF], f16, tag=f"sin{nb}", name=f"sin{nb}")
                nc.sync.dma_start(c[:], cosj[nb * P:(nb + 1) * P, :])
                nc.sync.dma_start(s[:], sinj[nb * P:(nb + 1) * P, :])
                cos_t[nb], sin_t[nb] = c, s
            load_trig(0)
            load_trig(1)
            rb_t = row_tiles(rb, nc.sync, "rb")
            rot_t = row_tiles(rot, nc.sync, "rot")
            load_trig(2)
            load_trig(3)
            dtw_t = load_consolidated(dtw, nc.sync, "dtw")
            ctr_t = load_consolidated(ctr, nc.sync, "ctr")
            ctin_t = load_consolidated(ctin, nc.sync, "ctin")

            # scalar: only the pre-first-evac loads
            def load_blocked(dram, eng, pfx, nb):
                t = cp.tile([P, IN], f16, tag=f"{pfx}{nb}", name=f"{pfx}{nb}")
                eng.dma_start(t[:], dram[:, nb * IN:(nb + 1) * IN])
                return t
            btrP_t = [load_blocked(btrP, nc.scalar, "btrP", 0)]
            btiP_t = [load_blocked(btiP, nc.scalar, "btiP", 0)]
            xh1a = [cp.tile([P, HH], f16, tag=f"xh1a{i}", name=f"xh1a{i}")
                    for i in range(2)]
            nc.scalar.dma_start(xh1a[0][:], xT[0:P, HH:HF])
            nc.scalar.dma_start(xh1a[1][:], xT[P:2 * P, HH:HF])
            btrP_t += [btrPr[:, (nb - 1) * IN:nb * IN] for nb in range(1, NB)]
            btiP_t += [btiPr[:, (nb - 1) * IN:nb * IN] for nb in range(1, NB)]

            def xv(ib, h):
                """x^T [P, HH] view for input block ib, half h."""
                if h == 0:
                    return xth0s[ib]
                if ib < 2:
                    return xh1a[ib][:]
                return xth1t[:, (ib - 2) * HH:(ib - 1) * HH]

            # ---------------- PE building blocks ----------------
            bus = {}

            def input_gemm(nb):
                for pi, wt in enumerate((btrP_t[nb], btiP_t[nb])):
                    bu = wkp.tile([P, HF], f16, tag=f"bu{pi}", bufs=2,
                                  name=f"bu{pi}_{nb}")
                    ps = [bp.tile([P, HH], f32, tag="ps", name=f"bups{h}")
                          for h in range(2)]
                    for ib in range(IBN):
                        for h in range(2):
                            nc.tensor.matmul(
                                ps[h][:], wt[:, ib * P:(ib + 1) * P], xv(ib, h),
                                start=(ib == 0), stop=(ib == IBN - 1))
                    for h in range(2):
                        nc.scalar.copy(bu[:, h * HH:(h + 1) * HH], ps[h][:])
                    bus[(nb, pi)] = bu

            # W-GEMM (flipped): w[i-block, n] = sum_s xw[s, i] * V[s, n]
            # planes: 0 = wre, 1 = wim, 2 = -wim (negated evac copy)
            wts = {}

            def w_gemm(pi):
                vt = (vre_t, vim_t)[pi]
                for ib in range(IBN):
                    ps = bp.tile([P, IN], f32, tag="ps", name="wps")
                    for sb in range(WB):
                        nc.tensor.matmul(
                            ps[:], xw_t[sb][:, ib * P:(ib + 1) * P], vt[sb][:],
                            start=(sb == 0), stop=(sb == WB - 1))
                    w = wkp.tile([P, IN], f16, tag=f"w{pi}{ib}", name=f"w{pi}{ib}")
                    nc.scalar.copy(w[:], ps[:])
                    wts[(pi, ib)] = w
                    if pi == 1:
                        wn = wkp.tile([P, IN], f16, tag=f"w2{ib}", name=f"w2{ib}")
                        nc.scalar.mul(wn[:], ps[:], -1.0)
                        wts[(2, ib)] = wn

            inits = {}

            def diag_carry(nb):
                """a_re/a_im via diag-block MMs + masked STT reduce + rotate."""
                nsl = slice(nb * P, (nb + 1) * P)
                bsl = lambda t, ib: t[:, ib * P:(ib + 1) * P]
                aps = {}
                for name in ("are", "aim"):
                    aps[name] = bp.tile([P, HH], f32, tag="ps", name=name)
                # weight-stationary: btrP feeds both planes; a_re's minus sign
                # rides the negated wim copy (plane 2)
                for ib in range(IBN):
                    nc.tensor.matmul(aps["are"][:, 0:P], bsl(btrP_t[nb], ib),
                                     wts[(0, ib)][:, nsl],
                                     start=(ib == 0), stop=False,
                                     skip_group_check=True)
                    nc.tensor.matmul(aps["aim"][:, 0:P], bsl(btrP_t[nb], ib),
                                     wts[(1, ib)][:, nsl],
                                     start=(ib == 0), stop=False,
                                     skip_group_check=True)
                for ib in range(IBN):
                    nc.tensor.matmul(aps["are"][:, 0:P], bsl(btiP_t[nb], ib),
                                     wts[(2, ib)][:, nsl],
                                     start=False, stop=(ib == IBN - 1),
                                     skip_group_check=True)
                    nc.tensor.matmul(aps["aim"][:, 0:P], bsl(btiP_t[nb], ib),
                                     wts[(0, ib)][:, nsl],
                                     start=False, stop=(ib == IBN - 1),
                                     skip_group_check=True)
                a_re = cp.tile([P, 1], f32, tag=f"are{nb}", name=f"are{nb}")
                a_im = cp.tile([P, 1], f32, tag=f"aim{nb}", name=f"aim{nb}")
                dre = wkp.tile([P, P], f16, tag="dre", name="dre")
                dim = wkp.tile([P, P], f16, tag="dim", name="dim")
                nc.vector.scalar_tensor_tensor(
                    dre[:], aps["are"][:, 0:P], 1.0, eye_t[:], AOP.bypass,
                    AOP.mult, accum_out=a_re[:])
                nc.vector.scalar_tensor_tensor(
                    dim[:], aps["aim"][:, 0:P], 1.0, eye_t[:], AOP.bypass,
                    AOP.mult, accum_out=a_im[:])
                # init = e^{i theta} * a
                i_re = cp.tile([P, 1], f32, tag=f"ire{nb}", name=f"ire{nb}")
                i_im = cp.tile([P, 1], f32, tag=f"iim{nb}", name=f"iim{nb}")
                u_re = cp.tile([P, 1], f32, tag=f"ure{nb}", name=f"ure{nb}")
                u_im = cp.tile([P, 1], f32, tag=f"uim{nb}", name=f"uim{nb}")
                nc.scalar.mul(u_re[:], a_re[:], rot_t[nb][:, 0:1])
                nc.vector.scalar_tensor_tensor(
                    i_re[:], a_im[:], rot_t[nb][:, 1:2], u_re[:], AOP.mult, AOP.add)
                nc.scalar.mul(u_im[:], a_im[:], rot_t[nb][:, 0:1])
                nc.vector.scalar_tensor_tensor(
                    i_im[:], a_re[:], rot_t[nb][:, 2:3], u_im[:], AOP.mult, AOP.add)
                inits[nb] = (i_re, i_im)

            # ---------------- DVE building blocks ----------------
            es = {}

            def twist(nb, h):
                hs = slice(h * HH, (h + 1) * HH)
                ct, st = cos_t[nb], sin_t[nb]
                br, bi = bus[(nb, 0)], bus[(nb, 1)]
                if h == 0:
                    e_re = wkp.tile([P, HF], f16, tag="er", bufs=4, name=f"er{nb}")
                    e_im = wkp.tile([P, HF], f16, tag="ei", bufs=4, name=f"ei{nb}")
                    es[nb] = (e_re, e_im)
                e_re, e_im = es[nb]
                p1 = wkp.tile([P, HH], f16, tag="p1", bufs=2, name="p1")
                p2 = wkp.tile([P, HH], f16, tag="p2", bufs=2, name="p2")
                nc.vector.tensor_tensor(p1[:], ct[:, hs], br[:, hs], AOP.mult)
                nc.vector.tensor_tensor(p2[:], st[:, hs], bi[:, hs], AOP.mult)
                nc.vector.tensor_tensor(e_re[:, hs], p1[:], p2[:], AOP.add)
                nc.vector.tensor_tensor(p1[:], ct[:, hs], bi[:, hs], AOP.mult)
                nc.vector.tensor_tensor(p2[:], st[:, hs], br[:, hs], AOP.mult)
                nc.vector.tensor_tensor(e_im[:, hs], p1[:], p2[:], AOP.subtract)

            ss = {}

            def scan(nb, h):
                """Full-length scans emitted at h==0 (FD=1024 runs ~2.02
                cyc/elem vs ~2.1 at FD=512 — one op per plane is cheaper)."""
                if h == 1:
                    return
                i_re, i_im = inits[nb]
                e_re, e_im = es[nb]
                s_re = wkp.tile([P, HF], f16, tag="sr", bufs=4, name=f"sr{nb}")
                s_im = wkp.tile([P, HF], f16, tag="si", bufs=4, name=f"si{nb}")
                ss[nb] = (s_re, s_im)
                rbb = rb_t[nb][:, 0:1].broadcast_to((P, HF))
                nc.vector.tensor_tensor_scan(
                    s_re[:], rbb, e_re[:], i_re[:, 0:1], AOP.mult, AOP.add)
                nc.vector.tensor_tensor_scan(
                    s_im[:], rbb, e_im[:], i_im[:, 0:1], AOP.mult, AOP.add)

            hhs = {}

            def untwist(nb, h):
                hs = slice(h * HH, (h + 1) * HH)
                s_re, s_im = ss[nb]
                ct, st = cos_t[nb], sin_t[nb]
                if h == 0:
                    hhr = wkp.tile([P, HF], f16, tag="hhr", bufs=4, name=f"hhr{nb}")
                    hhi = wkp.tile([P, HF], f16, tag="hhi", bufs=4, name=f"hhi{nb}")
                    hhs[nb] = (hhr, hhi)
                hhr, hhi = hhs[nb]
                q1 = wkp.tile([P, HH], f16, tag="q1", bufs=2, name="q1")
                q2 = wkp.tile([P, HH], f16, tag="q2", bufs=2, name="q2")
                q3 = wkp.tile([P, HH], f16, tag="q3", bufs=2, name="q3")
                q4 = wkp.tile([P, HH], f16, tag="q4", bufs=2, name="q4")
                nc.vector.tensor_tensor(q1[:], ct[:, hs], s_re[:, hs], AOP.mult)
                nc.vector.tensor_tensor(q2[:], st[:, hs], s_im[:, hs], AOP.mult)
                nc.vector.tensor_tensor(hhr[:, hs], q1[:], q2[:], AOP.subtract)
                nc.vector.tensor_tensor(q3[:], ct[:, hs], s_im[:, hs], AOP.mult)
                nc.vector.tensor_tensor(q4[:], st[:, hs], s_re[:, hs], AOP.mult)
                nc.vector.tensor_tensor(hhi[:, hs], q3[:], q4[:], AOP.add)

            # ---------------- emission ----------------
            # PE: in0-2, W, diag0 early (carry ready as twists end), in3 +
            # remaining diags after.  DVE: twists interleaved so a late bu3
            # never blocks ready scans; h1-untwists deferred so the h0 output
            # groups close (and store) an untwist-phase earlier.
            input_gemm(0)
            twist(0, 0)
            twist(0, 1)
            input_gemm(1)
            twist(1, 0)
            twist(1, 1)
            w_gemm(0)
            input_gemm(2)
            twist(2, 0)
            twist(2, 1)
            w_gemm(1)
            diag_carry(0)
            input_gemm(3)
            scan(0, 0)
            scan(0, 1)
            untwist(0, 0)
            twist(3, 0)
            twist(3, 1)
            diag_carry(1)
            scan(1, 0)
            scan(1, 1)
            untwist(1, 0)
            diag_carry(2)
            diag_carry(3)

            # output groups + D@x fill
            groups = {}
            for h in range(2):
                for ob in range(OBN):
                    gi = h * OBN + ob
                    pool = yp if gi < 5 else bp
                    tag = "yps" if gi < 5 else "ps"
                    groups[(h, ob)] = pool.tile([P, HH], f32, tag=tag, name="yps")
            for ob in range(OBN):
                osl = slice(ob * P, (ob + 1) * P)
                for ib in range(IBN):
                    for h in range(2):
                        nc.tensor.matmul(
                            groups[(h, ob)][:], dtw_t[ib][:, osl], xv(ib, h),
                            start=(ib == 0), stop=False)

            def store_group(h, ob):
                osl = slice(ob * P, (ob + 1) * P)
                hsl = slice(h * HH, (h + 1) * HH)
                ysb = wkp.tile([P, HH], f16, tag="ysb", bufs=4, name="ysb")
                nc.scalar.copy(ysb[:], groups[(h, ob)][:])
                eng = nc.sync if ob % 2 == 0 else nc.gpsimd
                eng.dma_start(yT[osl, hsl], ysb[:])

            def c_mms(nb, h):
                hhr, hhi = hhs[nb]
                last = nb == NB - 1
                for ob in range(OBN):
                    osl = slice(ob * P, (ob + 1) * P)
                    for wi, (wt, m) in enumerate(
                            ((ctr_t[nb][:, osl], hhr), (ctin_t[nb][:, osl], hhi))):
                        nc.tensor.matmul(
                            groups[(h, ob)][:], wt, m[:, h * HH:(h + 1) * HH],
                            start=False, stop=(last and wi == 1))
                    if last:
                        store_group(h, ob)

            scan(2, 0)
            scan(2, 1)
            untwist(2, 0)
            scan(3, 0)
            scan(3, 1)
            untwist(3, 0)
            c_mms(0, 0)
            c_mms(1, 0)
            c_mms(2, 0)
            c_mms(3, 0)
            untwist(0, 1)
            c_mms(0, 1)
            untwist(1, 1)
            c_mms(1, 1)
            untwist(2, 1)
            c_mms(2, 1)
            untwist(3, 1)
            c_mms(3, 1)

    nc.compile()
    nc.finalize()
    _CACHE["nc"] = nc
    return nc


def _host_prep(x, nu_log, theta_log, gamma_log, B_re, B_im, C_re, C_im, D):
    f64 = np.float64
    nu = np.asarray(nu_log, f64)
    th = np.asarray(theta_log, f64)
    gl = np.asarray(gamma_log, f64)
    r = np.exp(-np.exp(nu))
    theta = np.exp(th)
    gamma = np.exp(gl)

    gbr = gamma[:, None] * np.asarray(B_re, f64)   # [n, i]
    gbi = gamma[:, None] * np.asarray(B_im, f64)

    def blocked(w_ni):
        """[n, i] -> [p, (nb, ib, nl)] with p = i within block."""
        wt = np.ascontiguousarray(w_ni.T)  # [i, n]
        b = wt.reshape(IBN, P, NB, P)      # (ib, p, nb, nl)
        return np.ascontiguousarray(b.transpose(1, 2, 0, 3).reshape(P, NB * IN)
                                    ).astype(np.float16)

    shared = {
        "btrP": blocked(gbr),
        "btiP": blocked(gbi),
        "ctr": np.ascontiguousarray(np.asarray(C_re, f64).T).astype(np.float16),
        "ctin": np.ascontiguousarray((-np.asarray(C_im, f64)).T).astype(np.float16),
        "dtw": np.ascontiguousarray(np.asarray(D, f64).T).astype(np.float16),
        "eye": np.eye(P, dtype=np.float16),
    }
    j = np.arange(HF, dtype=f64)
    ang = theta[:, None] * j[None, :]
    shared["cosj"] = np.cos(ang).astype(np.float16)
    shared["sinj"] = np.sin(ang).astype(np.float16)
    # V = lam^{511-s} over the LAST WCON steps of the first half, [s, n]
    e = (WCON - 1) - np.arange(WCON, dtype=f64)
    mag = np.exp(np.log(r)[:, None] * e[None, :])
    angv = theta[:, None] * e[None, :]
    shared["vre"] = np.ascontiguousarray((mag * np.cos(angv)).T).astype(np.float16)
    shared["vim"] = np.ascontiguousarray((mag * np.sin(angv)).T).astype(np.float16)
    shared["rb"] = np.ascontiguousarray(r[:, None].astype(np.float32))
    shared["rot"] = np.stack(
        [np.cos(theta), -np.sin(theta), np.sin(theta)], axis=1).astype(np.float32)

    x = np.asarray(x, np.float32)
    in_maps = []
    for b in range(BATCH):
        m = dict(shared)
        m["xT"] = np.ascontiguousarray(x[b, HF:].T).astype(np.float16)
        m["xw"] = np.ascontiguousarray(x[b, HF - WCON:HF]).astype(np.float16)
        in_maps.append(m)
    return in_maps


def _run(in_maps, trace=False):
    nc = _build_nc()
    return run_bass_kernel_spmd(nc, in_maps, core_ids=list(range(BATCH)), trace=trace)


def kernel(**inputs):
    in_maps = _host_prep(**inputs)
    res = _run(in_maps, trace=False)
    y = np.stack([np.ascontiguousarray(res.results[b]["yT"].T) for b in range(BATCH)])
    return y.astype(np.float32)


def kernel_traced(**inputs):
    """Like kernel() but returns (y, exec_time_ns). Used by test.py."""
    in_maps = _host_prep(**inputs)
    res = _run(in_maps, trace=True)
    y = np.stack([np.ascontiguousarray(res.results[b]["yT"].T) for b in range(BATCH)])
    return y.astype(np.float32), res.exec_time_ns
